# revision 44
# baseline (speedup 1.0000x reference)
"""Trainium2 Bass kernel for the CIGAR GNN message-passing model (v4).

Data-parallel over batch across 8 NeuronCores (512 rows/core). Bottleneck
analysis of v3 (484us): GPSIMD SWDGE descriptor generation dominated, but the
4 gather queues were idle ~65% of the time — launch pacing (engine-stream
blocking on indirect DMAs, consumer-gated pool sems, long consume chains) was
the limit, not raw descriptor rate. v4 restructures for launch-stream flow:

  - GNN gathers use dma_gather(transpose=True): each 256B mem row lands as a
    COLUMN, so the per-tile chain is just matmul(lhsT=xT_tile, rhs=Wstack) ->
    PSUM [nbr, feat] -> tanh -> selector matmul. No tensor transposes, no wide
    DVE copies (which also contended with GPSIMD on the shared SBUF port).
  - streams are padded with -1: the gather ucode trims trailing negative
    indices, so capacity padding costs zero descriptors. Generous capacity
    (adgroup 14 tiles/window @ +7sigma, gnn 11 tiles/shard @ +8sigma) removes
    the spill machinery entirely. First-cycle pool buffers use 0-padding
    instead so no stale SBUF (NaN/Inf bit patterns) ever reaches a matmul.
  - user/item singles: one 4-offset indirect DMA per bt from a concatenated
    f32 table (was 6 separate indirects blocking the GPSIMD stream mid-wave).
  - counts / cate-table are stored pre-transposed in DRAM so their loads are
    1 packet per partition instead of 1 packet per 128B row.
  - launch stream: strict round-robin over the 4 SWDGE queues, emitted in
    launch-only phases that run ~1.5 bt ahead of the consume phases.

fp8(e4m3) lhsT x bf16 rhs matmuls are exact here (selector weights are small
int counts).
"""

import numpy as np

import concourse.bass as bass
import concourse.bacc as bacc
import concourse.mybir as mybir
import concourse.tile as tile
from concourse.bass_utils import run_bass_kernel_spmd
from concourse.masks import make_identity

NC = 8
B, S, N, D, G = 4096, 200, 64, 32, 64
BC = B // NC  # 512
NBT = BC // 128  # 4
VM = 200000
SHA = 25088  # adgroup shard width (int16 range)
NSH_A = 4
SHG = 28672  # mem shard width
NSH_G = 7
NW = 4  # 32-batch windows per bt
TW = 14  # adgroup tiles per (window, shard): capacity 1792 (mean ~1551)
LW = TW * 128
LA = NW * LW  # 7168 slots per (bt, shard)
TG = 11  # gnn tiles per (bt, shard): capacity 1408 (mean ~1142)
LG = TG * 128
NCATE = 79  # cate tile-rows (79*128 = 10112 >= 10001)
V1P = NCATE * 128
NQ = 4
ABUF = 18  # agp gather-dest pool depth (first-cycle zero-pad bookkeeping)
GBUF = 14  # ggp gather-dest pool depth
NTAB = 210000  # concat singles table: ut0(50k) ut1(50k) it0(100k) it1(10k)

F32 = mybir.dt.float32
BF16 = mybir.dt.bfloat16
FP8 = mybir.dt.float8e4
I16 = mybir.dt.int16
I32 = mybir.dt.int32

import os as _os
PARTS = _os.environ.get("KPARTS", "aucgm")
DEBUG = bool(_os.environ.get("KDEBUG"))
_CACHE = {}


def _build(has_bias=False, acnt=None, gcnt=None):
    # acnt[(bt, w, sh)] / gcnt[(bt, sh)]: per-launch gather lengths
    # (max over cores, rounded up to 128) baked in at compile time.
    if acnt is None:
        acnt = {(bt, w, sh): LW for bt in range(NBT) for w in range(NW) for sh in range(NSH_A)}
    if gcnt is None:
        gcnt = {(bt, sh): LG for bt in range(NBT) for sh in range(NSH_G)}
    nc = bacc.Bacc(None, target_bir_lowering=False, num_swdge_queues=NQ)

    # ---- DRAM inputs ----
    tab0b = nc.dram_tensor("tab0b", [NSH_A * SHA, 128], BF16, kind="ExternalInput")
    mem01b = nc.dram_tensor("mem01b", [NSH_G * SHG, 128], BF16, kind="ExternalInput")
    taball = nc.dram_tensor("taball", [NTAB, D], F32, kind="ExternalInput")
    t1rest = nc.dram_tensor("t1rest", [128, NCATE * D], BF16, kind="ExternalInput")
    countst = nc.dram_tensor("countst", [128, NBT * V1P], FP8, kind="ExternalInput")
    iui = nc.dram_tensor("iui", [128, NBT * 4], I32, kind="ExternalInput")
    aidx = nc.dram_tensor("aidx", [NBT * NSH_A * 128, LA // 16], I16, kind="ExternalInput")
    asel = nc.dram_tensor("asel", [NBT * NSH_A * 128, NW * TW * 32], FP8, kind="ExternalInput")
    gidx = nc.dram_tensor("gidx", [NBT * NSH_G * 128, LG // 16], I16, kind="ExternalInput")
    gsel = nc.dram_tensor("gsel", [NBT * NSH_G * 128, TG * 128], FP8, kind="ExternalInput")
    invseq = nc.dram_tensor("invseq", [128, NBT], F32, kind="ExternalInput")
    invn05d = nc.dram_tensor("invn05", [128, NBT], F32, kind="ExternalInput")
    wstack = nc.dram_tensor("wstack", [128, 128], BF16, kind="ExternalInput")
    bstack = nc.dram_tensor("bstack", [1, 128], BF16, kind="ExternalInput")
    w1t = nc.dram_tensor("w1t", [320, 256], F32, kind="ExternalInput")
    b1d = nc.dram_tensor("b1", [256], F32, kind="ExternalInput")
    w2t = nc.dram_tensor("w2t", [256, 128], F32, kind="ExternalInput")
    b2d = nc.dram_tensor("b2", [128], F32, kind="ExternalInput")
    w3t = nc.dram_tensor("w3t", [128, 1], F32, kind="ExternalInput")
    b3d = nc.dram_tensor("b3", [1], F32, kind="ExternalInput")
    out = nc.dram_tensor("out", [BC], F32, kind="ExternalOutput")
    warm = nc.dram_tensor("warm", [1, 4], BF16, kind="ExternalOutput")
    dbg = {}
    if DEBUG:
        for nm in ("dbgU", "dbgI", "dbgM", "dbgG"):
            dbg[nm] = nc.dram_tensor(nm, [BC, 64], F32, kind="ExternalOutput")
        dbg["dbgX"] = nc.dram_tensor("dbgX", [128, TG * 128], F32, kind="ExternalOutput")

    # Queue numbers are reassigned after tile scheduling (see below): Pool
    # DMAs rotate over 8 DMASW sem lanes in SCHEDULED order, and correctness
    # requires each lane to carry a single queue. The emission-time value is
    # a placeholder.
    def q():
        return 0

    with tile.TileContext(nc) as tc:
        with (
            tc.tile_pool(name="const", bufs=1) as cpool,
            tc.tile_pool(name="sb", bufs=2) as sb,
            tc.tile_pool(name="idx", bufs=6) as idxp,
            tc.tile_pool(name="ag", bufs=ABUF) as agp,
            tc.tile_pool(name="gg", bufs=GBUF) as ggp,
            tc.tile_pool(name="sel", bufs=6) as selp,
            tc.tile_pool(name="cnt", bufs=3) as cntp,
            tc.tile_pool(name="x", bufs=6) as xp,
            tc.tile_pool(name="mlp", bufs=2) as mlpp,
            tc.tile_pool(name="pch", bufs=2, space="PSUM") as pch,
            tc.tile_pool(name="pc2", bufs=2, space="PSUM") as pc2,
            tc.tile_pool(name="pga", bufs=2, space="PSUM") as pga,
            tc.tile_pool(name="pms", bufs=2, space="PSUM") as pms,
        ):
            # ---- constants ----
            identf = cpool.tile([128, 128], F32)
            make_identity(nc, identf[:])
            identb = cpool.tile([128, 128], BF16)
            make_identity(nc, identb[:])
            wst = cpool.tile([128, 128], BF16)
            nc.sync.dma_start(out=wst[:], in_=wstack[:])
            if has_bias:
                bstk = cpool.tile([1, 128], BF16, tag="bstk", name="bstk")
                nc.sync.dma_start(out=bstk[:], in_=bstack[:])
                ones1 = cpool.tile([1, 128], BF16, tag="ones1", name="ones1")
                nc.vector.memset(ones1[:], 1.0)
            invs = cpool.tile([128, NBT], F32)
            nc.sync.dma_start(out=invs[:], in_=invseq[:])
            invn_t = cpool.tile([128, NBT], F32, tag="invn05", name="invn05")
            nc.sync.dma_start(out=invn_t[:], in_=invn05d[:])
            w1ts = [cpool.tile([128, 256], F32, tag=f"w1t{k}", name=f"w1t{k}") for k in range(3)]
            for k in range(3):
                lo, hi = k * 128, min((k + 1) * 128, 320)
                nc.sync.dma_start(out=w1ts[k][: hi - lo, :], in_=w1t[lo:hi, :])
            w2ts = [cpool.tile([128, 128], F32, tag=f"w2t{k}", name=f"w2t{k}") for k in range(2)]
            for k in range(2):
                nc.sync.dma_start(out=w2ts[k][:], in_=w2t[k * 128 : (k + 1) * 128, :])
            w3ts = cpool.tile([128, 1], F32)
            nc.sync.dma_start(out=w3ts[:], in_=w3t[:])
            b1s = [cpool.tile([128, 1], F32, tag=f"b1{k}", name=f"b1{k}") for k in range(2)]
            for k in range(2):
                nc.sync.dma_start(out=b1s[k][:], in_=b1d[k * 128 : (k + 1) * 128, None])
            b2s = cpool.tile([128, 1], F32)
            nc.sync.dma_start(out=b2s[:], in_=b2d[:, None])
            b3s = cpool.tile([1, 1], F32)
            nc.sync.dma_start(out=b3s[:], in_=b3d[:, None])
            t1res = cpool.tile([128, NCATE * D], BF16)
            if "c" in PARTS:
                nc.scalar.dma_start(out=t1res[:], in_=t1rest[:])

            # warmup gather: absorb the cold-start ucode IRAM load (k=0, q0)
            if "a" in PARTS or "g" in PARTS:
                wit = cpool.tile([128, 8], I16, tag="warmidx", name="warmidx")
                nc.sync.dma_start(out=wit[:], in_=gidx[0:128, 0:8])
                wdest = cpool.tile([128, 128], BF16, tag="warmdest", name="warmdest")
                nc.gpsimd.dma_gather(
                    out_ap=wdest[:].rearrange("p (s e) -> p s e", e=128),
                    in_ap=mem01b[0:SHG, :],
                    idxs_ap=wit[:],
                    num_idxs=128,
                    num_idxs_reg=128,
                    elem_size=128,
                    single_packet=False,
                    queue_num=0,
                )
                nc.sync.dma_start(out=warm[:, :], in_=wdest[0:1, 0:4])

            # user/item singles: per-(bt, piece) [128,1]-offset indirects (the
            # HW DGE only supports one offset per partition row). Queues are
            # normalized post-schedule along with the gathers.
            iuit = cpool.tile([128, NBT * 4], I32, tag="iuit", name="iuit")
            nc.sync.dma_start(out=iuit[:], in_=iui[:])
            UIall = cpool.tile([128, NBT * 4 * D], F32, tag="UIall", name="UIall")
            if "u" not in PARTS:
                nc.vector.memset(UIall[:], 0.0)
            UIs = {bt: UIall[:, bt * 4 * D : (bt + 1) * 4 * D] for bt in range(NBT)}

            def eui(bt):
                # singles for this bt; spread out so the cores-0/1 descriptor
                # work interleaves with gather generation instead of blocking
                # the head of the kernel
                if "u" not in PARTS:
                    return
                for j in range(bt * 4, (bt + 1) * 4):
                    nc.gpsimd.indirect_dma_start(
                        out=UIall[:, j * D : (j + 1) * D],
                        out_offset=None,
                        in_=taball[:],
                        in_offset=bass.IndirectOffsetOnAxis(
                            ap=iuit[:, j : j + 1], axis=0
                        ),
                    )

            gdests, gsels = {}, {}
            adests, asels = {}, {}
            gnns = {}

            def eg(bt):
                """Launch GNN gathers for bt (7 shard streams)."""
                for sh in range(NSH_G if "g" in PARTS else 0):
                    r0 = (bt * NSH_G + sh) * 128
                    it = idxp.tile([128, LG // 16], I16, tag="gidx", bufs=14)
                    nc.sync.dma_start(out=it[:], in_=gidx[r0 : r0 + 128, :])
                    sl = selp.tile([128, TG * 128], FP8, tag="gsel", bufs=14)
                    nc.scalar.dma_start(out=sl[:], in_=gsel[r0 : r0 + 128, :])
                    gsels[(bt, sh)] = sl
                    X = ggp.tile([128, TG * 128], BF16, tag="gdest", bufs=GBUF)
                    ng = gcnt[(bt, sh)]
                    nc.gpsimd.dma_gather(
                        out_ap=X[:, : ng].rearrange("p (s e) -> p s e", e=128),
                        in_ap=mem01b[sh * SHG : (sh + 1) * SHG, :],
                        idxs_ap=it[:, : ng // 16],
                        num_idxs=ng,
                        num_idxs_reg=ng,
                        elem_size=128,
                        single_packet=False,
                        queue_num=q(),
                    )
                    gdests[(bt, sh)] = X

            def ea(bt):
                """Launch adgroup gathers for bt (4 windows x 4 shards)."""
                aits = []
                for sh in range(NSH_A if "a" in PARTS else 0):
                    r0 = (bt * NSH_A + sh) * 128
                    it = idxp.tile([128, LA // 16], I16, tag="aidx", bufs=8)
                    nc.sync.dma_start(out=it[:], in_=aidx[r0 : r0 + 128, :])
                    aits.append(it)
                    sl = selp.tile([128, NW * TW * 32], FP8, tag="asel", bufs=8)
                    nc.scalar.dma_start(out=sl[:], in_=asel[r0 : r0 + 128, :])
                    asels[(bt, sh)] = sl
                for w in range(NW if "a" in PARTS else 0):
                    for sh in range(NSH_A):
                        dest = agp.tile([128, TW * 128], BF16, tag="adest", bufs=ABUF)
                        na = acnt[(bt, w, sh)]
                        nc.gpsimd.dma_gather(
                            out_ap=dest[:, : na].rearrange("p (s e) -> p s e", e=128),
                            in_ap=tab0b[sh * SHA : (sh + 1) * SHA, :],
                            idxs_ap=aits[sh][
                                :, w * (LW // 16) : w * (LW // 16) + na // 16
                            ],
                            num_idxs=na,
                            num_idxs_reg=na,
                            elem_size=128,
                            single_packet=False,
                            queue_num=q(),
                        )
                        adests[(bt, w, sh)] = dest

            def cg(bt):
                """Consume GNN gathers -> gnn[bt] (tanh(W x) aggregated)."""
                gnn = sb.tile([128, G], F32, tag="gnn", name=f"gnn{bt}", bufs=2)
                gnns[bt] = gnn
                if "g" not in PARTS:
                    nc.vector.memset(gnn[:], 0.0)
                    return
                nt = sum(gcnt[(bt, sh)] // 128 for sh in range(NSH_G))
                gaccA = pga.tile([128, 128], F32, tag="gacc", name=f"gaccA{bt}")
                gaccB = pga.tile([128, 128], F32, tag="gacc", name=f"gaccB{bt}")
                lastA = ((nt - 1) // 2) * 2
                lastB = ((nt - 2) // 2) * 2 + 1
                k = 0
                for sh in range(NSH_G):
                    X, sl = gdests[(bt, sh)], gsels[(bt, sh)]
                    tg_sh = gcnt[(bt, sh)] // 128
                    for t0 in range(0, tg_sh, 4):
                        gw = min(4, tg_sh - t0)
                        # transpose gw tiles [pos, feat] -> [feat, pos]
                        xt_ps = pch.tile(
                            [128, 4 * 128], BF16, tag="pchain", name=f"xt{bt}_{k}"
                        )
                        for i in range(gw):
                            nc.tensor.transpose(
                                out=xt_ps[:, i * 128 : (i + 1) * 128],
                                in_=X[:, (t0 + i) * 128 : (t0 + i + 1) * 128],
                                identity=identb[:],
                            )
                        xt = xp.tile([128, 4 * 128], BF16, tag="xt", bufs=3)
                        nc.vector.tensor_copy(
                            out=xt[:, : gw * 128], in_=xt_ps[:, : gw * 128]
                        )
                        for i in range(gw):
                            ps = pc2.tile([128, 128], F32, tag="pc2", name=f"ps{bt}_{k}")
                            if has_bias:
                                nc.tensor.matmul(
                                    ps[:], lhsT=ones1[:], rhs=bstk[:],
                                    start=True, stop=False,
                                )
                            nc.tensor.matmul(
                                ps[:],
                                lhsT=xt[:, i * 128 : (i + 1) * 128],
                                rhs=wst[:],
                                start=not has_bias,
                                stop=True,
                            )
                            h = xp.tile([128, 128], BF16, tag="h", bufs=6)
                            nc.scalar.activation(
                                out=h[:], in_=ps[:],
                                func=mybir.ActivationFunctionType.Tanh,
                            )
                            gacc = gaccA if k % 2 == 0 else gaccB
                            nc.tensor.matmul(
                                gacc[:],
                                lhsT=sl[:, (t0 + i) * 128 : (t0 + i + 1) * 128],
                                rhs=h[:],
                                start=k < 2,
                                stop=k in (lastA, lastB),
                            )
                            k += 1
                nc.vector.tensor_copy(out=gnn[:], in_=gaccA[:, :G])
                nc.vector.tensor_tensor(
                    out=gnn[:], in0=gnn[:], in1=gaccA[:, G:], op=mybir.AluOpType.add
                )
                nc.vector.tensor_tensor(
                    out=gnn[:], in0=gnn[:], in1=gaccB[:, :G], op=mybir.AluOpType.add
                )
                nc.vector.tensor_tensor(
                    out=gnn[:], in0=gnn[:], in1=gaccB[:, G:], op=mybir.AluOpType.add
                )
                nc.vector.tensor_scalar_mul(gnn[:], gnn[:], invn_t[:, bt : bt + 1])

            ftbs = {}

            def caW(bt):
                """Windows + cate + M + Pp + gnn-independent feature pieces.

                Runs EARLY on the tensor stream so the adgroup dest pool
                frees before the next bt's launches need it."""
                UI = UIs[bt]
                U, I = UI[:, : 2 * D], UI[:, 2 * D :]

                ftb = [
                    sb.tile([128, 128], F32, tag="ftb0", name=f"ftb0_{bt}"),
                    sb.tile([128, 128], F32, tag="ftb1", name=f"ftb1_{bt}"),
                    sb.tile([64, 128], F32, tag="ftb2", name=f"ftb2_{bt}"),
                ]
                ftbs[bt] = ftb
                for pi, piece in ((0, U), (1, I)):
                    p_ps = pms.tile([64, 128], F32, tag="pmisc", name=f"pt{bt}_{pi}")
                    nc.tensor.transpose(out=p_ps[:], in_=piece[:], identity=identf[:])
                    slab, row = divmod(pi * 64, 128)
                    nc.vector.tensor_copy(out=ftb[slab][row : row + 64, :], in_=p_ps[:])

                # window matmuls -> M
                M = sb.tile([128, 2 * D], F32, tag="M", name=f"M{bt}")
                if "a" not in PARTS:
                    nc.vector.memset(M[:], 0.0)
                for w in range(NW if "a" in PARTS else 0):
                    wps = pms.tile([32, 32], F32, tag="pmisc", name=f"wps{bt}_{w}")
                    nmm = sum(acnt[(bt, w, sh)] // 128 for sh in range(NSH_A))
                    k = 0
                    for sh in range(NSH_A):
                        sl = asels[(bt, sh)]
                        dest = adests[(bt, w, sh)]
                        for t in range(acnt[(bt, w, sh)] // 128):
                            nc.tensor.matmul(
                                wps[:],
                                lhsT=sl[:, (w * TW + t) * 32 : (w * TW + t + 1) * 32],
                                rhs=dest[:, t * 128 : t * 128 + 32],
                                start=(k == 0),
                                stop=(k == nmm - 1),
                            )
                            k += 1
                    nc.vector.tensor_copy(out=M[32 * w : 32 * w + 32, :D], in_=wps[:])

                # cate seq-sum (count-matmul against the resident cate table)
                cps = pms.tile([128, 32], F32, tag="pmisc", name=f"cps{bt}")
                for cg_i in range(5 if "c" in PARTS else 0):
                    c0, c1 = cg_i * 16, min((cg_i + 1) * 16, NCATE)
                    cs = cntp.tile([128, 16 * 128], FP8, tag="cnt", bufs=3)
                    nc.scalar.dma_start(
                        out=cs[:, : (c1 - c0) * 128],
                        in_=countst[:, bt * V1P + c0 * 128 : bt * V1P + c1 * 128],
                    )
                    for c in range(c0, c1):
                        nc.tensor.matmul(
                            cps[:],
                            lhsT=cs[:, (c - c0) * 128 : (c - c0 + 1) * 128],
                            rhs=t1res[:, c * D : (c + 1) * D],
                            start=(c == 0),
                            stop=(c == NCATE - 1),
                        )

                if "c" in PARTS:
                    nc.vector.tensor_copy(out=M[:, D:], in_=cps[:])
                else:
                    nc.vector.memset(M[:, D:], 0.0)
                nc.vector.tensor_scalar_mul(M[:], M[:], invs[:, bt : bt + 1])

                Pp = sb.tile([128, 2 * D], F32, tag="Pp")
                nc.vector.tensor_tensor(
                    out=Pp[:], in0=I[:], in1=M[:], op=mybir.AluOpType.mult
                )
                if DEBUG:
                    bsl = slice(bt * 128, (bt + 1) * 128)
                    for nm, tl in (("dbgU", U), ("dbgI", I), ("dbgM", M)):
                        nc.sync.dma_start(out=dbg[nm][bsl, :], in_=tl[:])

                # transpose remaining feature pieces (M, Pp) into fT tiles
                for pi, piece in ((2, M), (3, Pp)):
                    p_ps = pms.tile([64, 128], F32, tag="pmisc", name=f"pt{bt}_{pi}")
                    nc.tensor.transpose(out=p_ps[:], in_=piece[:], identity=identf[:])
                    slab, row = divmod(pi * 64, 128)
                    nc.vector.tensor_copy(out=ftb[slab][row : row + 64, :], in_=p_ps[:])

            def caT(bt):
                """gnn feature piece + MLP + output for bt (after cg(bt))."""
                bsl = slice(bt * 128, (bt + 1) * 128)
                gnn = gnns[bt]
                ftb = ftbs[bt]
                if DEBUG:
                    nc.sync.dma_start(out=dbg["dbgG"][bsl, :], in_=gnn[:])
                p_ps = pms.tile([64, 128], F32, tag="pmisc", name=f"pt{bt}_4")
                nc.tensor.transpose(out=p_ps[:], in_=gnn[:], identity=identf[:])
                slab, row = divmod(4 * 64, 128)
                nc.vector.tensor_copy(out=ftb[slab][row : row + 64, :], in_=p_ps[:])

                # per-bt MLP slice
                h1s = []
                for m in range(2):
                    h1_ps = pms.tile([128, 128], F32, tag="pmisc", name=f"h1ps{bt}_{m}")
                    for oi, kk in enumerate((0, 2, 1)):
                        kp = 128 if kk < 2 else 64
                        nc.tensor.matmul(
                            h1_ps[:],
                            lhsT=w1ts[kk][:kp, m * 128 : (m + 1) * 128],
                            rhs=ftb[kk][:kp, :],
                            start=(oi == 0),
                            stop=(oi == 2),
                        )
                    h1 = mlpp.tile([128, 128], F32, tag="h1", name=f"h1_{bt}_{m}")
                    nc.scalar.activation(
                        out=h1[:], in_=h1_ps[:],
                        func=mybir.ActivationFunctionType.Relu, bias=b1s[m][:, 0:1],
                    )
                    h1s.append(h1)
                h2_ps = pms.tile([128, 128], F32, tag="pmisc", name=f"h2ps{bt}")
                for m in range(2):
                    nc.tensor.matmul(
                        h2_ps[:], lhsT=w2ts[m][:], rhs=h1s[m][:],
                        start=(m == 0), stop=(m == 1),
                    )
                h2 = mlpp.tile([128, 128], F32, tag="h2", name=f"h2_{bt}")
                nc.scalar.activation(
                    out=h2[:], in_=h2_ps[:],
                    func=mybir.ActivationFunctionType.Relu, bias=b2s[:, 0:1],
                )
                lg_ps = pms.tile([1, 128], F32, tag="pmisc", name=f"lgps{bt}")
                nc.tensor.matmul(lg_ps[:], lhsT=w3ts[:], rhs=h2[:])
                lgt = mlpp.tile([1, 128], F32, tag="lg", name=f"lgt{bt}")
                nc.vector.tensor_scalar_add(lgt[:], lg_ps[:], b3s[:, 0:1])
                nc.sync.dma_start(out=out[None, bsl], in_=lgt[:])

            eg(0)
            eui(0)
            ea(0)
            eg(1)
            eui(1)
            caW(0)
            cg(0)
            ea(1)
            caT(0)
            eg(2)
            eui(2)
            caW(1)
            cg(1)
            ea(2)
            caT(1)
            eg(3)
            eui(3)
            caW(2)
            cg(2)
            ea(3)
            caT(2)
            cg(3)
            caW(3)
            caT(3)

    # Post-schedule queue assignment: the k-th Pool DMA (scheduled order)
    # uses DMASW lane k%8; each lane must stay on one SWDGE queue, so assign
    # queue = lane%4 to every Pool DMA (gathers via queue_num, indirect
    # copies via their queue name).
    pool_seq = []
    for bb in nc.m.functions[0].blocks:
        for inst in bb.instructions:
            nm = type(inst).__name__
            eng = getattr(inst, "engine", None)
            if nm == "InstDMAGatherAnt" or (
                nm == "InstDMACopy" and str(eng) == "EngineType.Pool"
            ):
                pool_seq.append((nm, inst))
    for k, (nm, inst) in enumerate(pool_seq):
        qq = (k % 8) % NQ
        if nm == "InstDMAGatherAnt":
            inst.queue_num = qq
        else:
            inst.queue = f"qPoolDynamic{qq or ''}"

    nc.compile()
    return nc


def _prep(inp):
    """Host-side input transforms -> per-core in_maps."""
    f32 = np.float32
    bf16 = mybir.dt.np(BF16)
    fp8 = mybir.dt.np(FP8)
    g = lambda k: np.asarray(inp[k])

    it0 = g("item_tab0").astype(f32)  # [100000, 32]
    it1 = g("item_tab1").astype(f32)  # [10000, 32]
    tab0b = np.zeros((NSH_A * SHA, 128), bf16)
    tab0b[: it0.shape[0], :D] = it0.astype(bf16)
    mem01 = np.concatenate([g("mem0"), g("mem1")], axis=1).astype(f32)
    mem01b = np.zeros((NSH_G * SHG, 128), bf16)
    mem01b[:VM] = mem01.astype(bf16)
    ut0 = np.asarray(g("user_tab0"), f32)
    ut1 = np.asarray(g("user_tab1"), f32)
    taball = np.zeros((NTAB, D), f32)
    taball[:50000] = ut0
    taball[50000:100000] = ut1
    taball[100000:200000] = it0
    taball[200000:210000] = it1
    tab1p = np.zeros((V1P, D), f32)
    tab1p[: it1.shape[0]] = it1
    t1rest = np.ascontiguousarray(
        tab1p.astype(bf16).reshape(NCATE, 128, D).transpose(1, 0, 2).reshape(128, NCATE * D)
    )

    wstack = np.zeros((128, 128), bf16)
    wstack[:G, :G] = g("W_agg0").T.astype(bf16)
    wstack[G:, G:] = g("W_agg1").T.astype(bf16)
    bstack = np.concatenate([g("b_agg0"), g("b_agg1")]).astype(f32)
    has_bias = bool(np.abs(bstack).max() > 0)
    w1t = np.ascontiguousarray(g("W1").T.astype(f32))
    w2t = np.ascontiguousarray(g("W2").T.astype(f32))
    w3t = np.ascontiguousarray(g("W3").T.astype(f32))
    b1 = g("b1").astype(f32); b2 = g("b2").astype(f32); b3 = g("b3").astype(f32)

    aseq = g("adgroup_id_seq").astype(np.int64)
    cseq = g("cate_id_seq").astype(np.int64)
    nbr = g("neighbor_ids").astype(np.int64)
    seq_mask = aseq != 0
    invseq_all = (1.0 / np.maximum(seq_mask.sum(-1), 1)).astype(f32)
    nmask = nbr != 0
    invn = (0.5 / np.maximum(nmask.sum(-1), 1)).astype(f32)

    def pack16(stream):
        # [L] -> [128, L//16]: idx k at [k%16, k//16], replicated x8
        w = stream.reshape(-1, 16).T.astype(np.int16)
        return np.tile(w, (8, 1))

    nl_a = {}  # (bt, w, sh) -> max nload over cores
    nl_g = {}  # (bt, sh) -> max nload over cores
    in_maps = []
    for c in range(NC):
        bs = slice(c * BC, (c + 1) * BC)
        a_c, c_c, m_c = aseq[bs], cseq[bs], seq_mask[bs]
        n_c = nbr[bs]
        invn_c = invn[bs]

        aidx_l = np.zeros((NBT * NSH_A * 128, LA // 16), np.int16)
        asel_l = np.zeros((NBT * NSH_A * 128, NW * TW * 32), fp8)
        gidx_l = np.zeros((NBT * NSH_G * 128, LG // 16), np.int16)
        gsel_l = np.zeros((NBT * NSH_G * 128, TG * 128), fp8)
        countst_l = np.zeros((128, NBT * V1P), fp8)
        iui_l = np.zeros((128, NBT * 4), np.int32)



        for bt in range(NBT):
            btsl = slice(bt * 128, (bt + 1) * 128)
            a = a_c[btsl]; cc = c_c[btsl]; mm = m_c[btsl]
            b_loc = np.repeat(np.arange(128), S)
            av = a.ravel(); mv = mm.ravel()
            b_m = b_loc[mv]; a_m = av[mv]
            sh_a = a_m // SHA; loc_a = a_m % SHA
            w_a = b_m // 32

            # adgroup streams: per (sh, w), dedup by loc with per-b counts
            for sh in range(NSH_A):
                stream4 = np.zeros(LA, np.int64)
                sel = np.zeros((128, NW * TW * 32), f32)
                for w in range(NW):
                    pick = (sh_a == sh) & (w_a == w)
                    ll, bb = loc_a[pick], b_m[pick] - 32 * w
                    key = ll * 32 + bb
                    uk, cnt = np.unique(key, return_counts=True)
                    lw, bw = uk // 32, uk % 32
                    uloc, inv = np.unique(lw, return_inverse=True)
                    nload = len(uloc)
                    assert nload <= LW, f"adgroup overflow {nload} > {LW}"
                    nl_a[(bt, w, sh)] = max(nl_a.get((bt, w, sh), 0), nload)
                    base = w * LW
                    stream4[base : base + nload] = uloc
                    stream4[base + nload : base + LW] = 0  # 0-pad (no trim)
                    pos = inv
                    sel[pos % 128, (w * TW + pos // 128) * 32 + bw] = cnt
                r0 = (bt * NSH_A + sh) * 128
                aidx_l[r0 : r0 + 128] = pack16(stream4)
                asel_l[r0 : r0 + 128] = sel.astype(fp8)

            # cate counts, pre-transposed: [128 p, (c,b)] = C[c*128+p, b]
            cm = cc.ravel()[mv]
            C = np.bincount(cm * 128 + b_m, minlength=V1P * 128).reshape(V1P, 128)
            countst_l[:, bt * V1P : (bt + 1) * V1P] = (
                C.reshape(NCATE, 128, 128).transpose(1, 0, 2).reshape(128, V1P).astype(fp8)
            )

            # GNN streams: per sh, dedup by loc with per-b weights
            nb = n_c[btsl]
            b_loc2 = np.repeat(np.arange(128), N)
            nv = nb.ravel()
            sh_g = nv // SHG; loc_g = nv % SHG
            for sh in range(NSH_G):
                pick = sh_g == sh
                ll, bb = loc_g[pick], b_loc2[pick]
                key = ll * 128 + bb
                uk, cnt = np.unique(key, return_counts=True)
                lw, bw = uk // 128, uk % 128
                wv = cnt * ((lw != 0) | (sh != 0))
                uloc, inv = np.unique(lw, return_inverse=True)
                nload = len(uloc)
                assert nload <= LG, f"gnn overflow {nload} > {LG}"
                nl_g[(bt, sh)] = max(nl_g.get((bt, sh), 0), nload)
                stream = np.zeros(LG, np.int64)  # 0-pad (no trim)
                stream[:nload] = uloc
                pos = inv
                sel = np.zeros((128, TG * 128), f32)
                sel[pos % 128, (pos // 128) * 128 + bw] = wv
                r0 = (bt * NSH_G + sh) * 128
                gidx_l[r0 : r0 + 128] = pack16(stream)
                gsel_l[r0 : r0 + 128] = sel.astype(fp8)

            # merged singles indices
            iui_l[:, bt * 4 + 0] = g("user_f0")[bs][btsl].astype(np.int32)
            iui_l[:, bt * 4 + 1] = 50000 + g("user_f1")[bs][btsl].astype(np.int32)
            iui_l[:, bt * 4 + 2] = 100000 + g("adgroup_id")[bs][btsl].astype(np.int32)
            iui_l[:, bt * 4 + 3] = 200000 + g("cate_id")[bs][btsl].astype(np.int32)

        in_maps.append(
            {
                "tab0b": tab0b, "mem01b": mem01b, "taball": taball,
                "t1rest": t1rest, "countst": countst_l, "iui": iui_l,
                "aidx": aidx_l, "gidx": gidx_l, "asel": asel_l, "gsel": gsel_l,
                "invseq": invseq_all[bs].reshape(NBT, 128).T.copy(),
                "invn05": invn_c.reshape(NBT, 128).T.copy().astype(f32),
                "wstack": wstack,
                "bstack": np.ascontiguousarray(bstack.astype(bf16)[None, :]),
                "w1t": w1t, "b1": b1, "w2t": w2t, "b2": b2, "w3t": w3t, "b3": b3,
            }
        )
    r128 = lambda n: max(128, -(-n // 128) * 128)
    acnt = {k: r128(v) for k, v in nl_a.items()}
    gcnt = {k: r128(v) for k, v in nl_g.items()}
    return in_maps, has_bias, acnt, gcnt


def kernel(**inputs) -> np.ndarray:
    in_maps, has_bias, acnt, gcnt = _prep(inputs)
    key = ("nc", has_bias, tuple(sorted(acnt.items())), tuple(sorted(gcnt.items())))
    if key not in _CACHE:
        _CACHE[key] = _build(has_bias, acnt, gcnt)
    nc = _CACHE[key]
    trace = bool(_os.environ.get("KERNEL_TRACE"))
    res = run_bass_kernel_spmd(nc, in_maps, list(range(NC)), trace=trace)
    _CACHE["last_result"] = res
    out = np.concatenate([res.results[c]["out"] for c in range(NC)])
    return out[:, None].astype(np.float32)


# revision 46
# speedup vs baseline: 1.0984x; 1.0984x over previous
"""Trainium2 Bass kernel for the CIGAR GNN message-passing model (v5).

Data-parallel over batch across 8 NeuronCores (512 rows/core). All bulk
gathers use the SWDGE dma_gather ucode (256B rows, int16 shard-local
indices, 4 queues). Main differences vs the v3 baseline (440-484us):

  - input-specialized compilation: _build() bakes per-launch gather lengths
    (max over the 8 cores, rounded to 128) as compile-time constants, so
    stream capacity padding (spill-free margins) costs no descriptors and
    no consumer matmul tiles. Host prep dedups (loc)-per-window streams.
  - GNN chain per 128-row tile: tensor-transpose -> PSUM bf16 -> copy ->
    matmul(lhsT=xT_tile, rhs=Wstack) -> PSUM [pos, feat_out] -> tanh ->
    fp8-selector matmul accumulate. One transpose per tile (v3 had two)
    and half the wide DVE copies (DVE contends with GPSIMD's SBUF port).
    (A dma_gather(transpose=True) variant avoided the transpose entirely
    but was nondeterministically corrupted under concurrency on HW - the
    xbar spray path appears unsafe with concurrent gathers; reverted.)
  - Pool-engine DMAs rotate over 8 DMASW sem lanes in SCHEDULED order and
    each lane must stay on one SWDGE queue; queue numbers (and the
    indirect copies' queue names) are reassigned post-scheduling to
    queue = lane%4. This also balances the 4 Q7 core pairs.
  - user/item/spill singles: 16 [128,1]-offset indirect DMAs from one
    concatenated f32 table (multi-offset indirects are not supported by
    the HW DGE walker, verified empirically).
  - counts / cate-table stored pre-transposed in DRAM: loads are 1 packet
    per partition instead of 1 packet per 128B row (halves HWDGE packets).
  - consume phases split (caW windows before the 77-tile GNN chain) so the
    adgroup dest pool frees before the next bt's launches need it; the
    last bt consumes its GNN chain during the final adgroup drain.

fp8(e4m3) lhsT x bf16 rhs matmuls are exact here (selector weights are small
int counts). HW exec ~418us vs ~440-484us for the v3 baseline.
"""

import numpy as np

import concourse.bass as bass
import concourse.bacc as bacc
import concourse.mybir as mybir
import concourse.tile as tile
from concourse.bass_utils import run_bass_kernel_spmd
from concourse.masks import make_identity

NC = 8
B, S, N, D, G = 4096, 200, 64, 32, 64
BC = B // NC  # 512
NBT = BC // 128  # 4
VM = 200000
SHA = 25088  # adgroup shard width (int16 range)
NSH_A = 4
SHG = 28672  # mem shard width
NSH_G = 7
NW = 4  # 32-batch windows per bt
TW = 14  # adgroup tiles per (window, shard): capacity 1792 (mean ~1551)
LW = TW * 128
LA = NW * LW  # 7168 slots per (bt, shard)
TG = 11  # gnn tiles per (bt, shard): capacity 1408 (mean ~1142)
LG = TG * 128
NCATE = 79  # cate tile-rows (79*128 = 10112 >= 10001)
V1P = NCATE * 128
NQ = 4
ABUF = 18  # agp gather-dest pool depth (first-cycle zero-pad bookkeeping)
GBUF = 14  # ggp gather-dest pool depth
NTAB = 210000  # concat singles table: ut0(50k) ut1(50k) it0(100k) it1(10k)

F32 = mybir.dt.float32
BF16 = mybir.dt.bfloat16
FP8 = mybir.dt.float8e4
I16 = mybir.dt.int16
I32 = mybir.dt.int32

import os as _os
PARTS = _os.environ.get("KPARTS", "aucgm")
DEBUG = bool(_os.environ.get("KDEBUG"))
_CACHE = {}


def _build(has_bias=False, acnt=None, gcnt=None):
    # acnt[(bt, w, sh)] / gcnt[(bt, sh)]: per-launch gather lengths
    # (max over cores, rounded up to 128) baked in at compile time.
    if acnt is None:
        acnt = {(bt, w, sh): LW for bt in range(NBT) for w in range(NW) for sh in range(NSH_A)}
    if gcnt is None:
        gcnt = {(bt, sh): LG for bt in range(NBT) for sh in range(NSH_G)}
    nc = bacc.Bacc(None, target_bir_lowering=False, num_swdge_queues=NQ)

    # ---- DRAM inputs ----
    tab0b = nc.dram_tensor("tab0b", [NSH_A * SHA, 128], BF16, kind="ExternalInput")
    mem01b = nc.dram_tensor("mem01b", [NSH_G * SHG, 128], BF16, kind="ExternalInput")
    taball = nc.dram_tensor("taball", [NTAB, D], F32, kind="ExternalInput")
    t1rest = nc.dram_tensor("t1rest", [128, NCATE * D], BF16, kind="ExternalInput")
    countst = nc.dram_tensor("countst", [128, NBT * V1P], FP8, kind="ExternalInput")
    iui = nc.dram_tensor("iui", [128, NBT * 4], I32, kind="ExternalInput")
    aidx = nc.dram_tensor("aidx", [NBT * NSH_A * 128, LA // 16], I16, kind="ExternalInput")
    asel = nc.dram_tensor("asel", [NBT * NSH_A * 128, NW * TW * 32], FP8, kind="ExternalInput")
    gidx = nc.dram_tensor("gidx", [NBT * NSH_G * 128, LG // 16], I16, kind="ExternalInput")
    gsel = nc.dram_tensor("gsel", [NBT * NSH_G * 128, TG * 128], FP8, kind="ExternalInput")
    invseq = nc.dram_tensor("invseq", [128, NBT], F32, kind="ExternalInput")
    invn05d = nc.dram_tensor("invn05", [128, NBT], F32, kind="ExternalInput")
    wstack = nc.dram_tensor("wstack", [128, 128], BF16, kind="ExternalInput")
    bstack = nc.dram_tensor("bstack", [1, 128], BF16, kind="ExternalInput")
    w1t = nc.dram_tensor("w1t", [320, 256], F32, kind="ExternalInput")
    b1d = nc.dram_tensor("b1", [256], F32, kind="ExternalInput")
    w2t = nc.dram_tensor("w2t", [256, 128], F32, kind="ExternalInput")
    b2d = nc.dram_tensor("b2", [128], F32, kind="ExternalInput")
    w3t = nc.dram_tensor("w3t", [128, 1], F32, kind="ExternalInput")
    b3d = nc.dram_tensor("b3", [1], F32, kind="ExternalInput")
    out = nc.dram_tensor("out", [BC], F32, kind="ExternalOutput")
    warm = nc.dram_tensor("warm", [1, 4], BF16, kind="ExternalOutput")
    dbg = {}
    if DEBUG:
        for nm in ("dbgU", "dbgI", "dbgM", "dbgG"):
            dbg[nm] = nc.dram_tensor(nm, [BC, 64], F32, kind="ExternalOutput")
        dbg["dbgX"] = nc.dram_tensor("dbgX", [128, TG * 128], F32, kind="ExternalOutput")

    # Queue numbers are reassigned after tile scheduling (see below): Pool
    # DMAs rotate over 8 DMASW sem lanes in SCHEDULED order, and correctness
    # requires each lane to carry a single queue. The emission-time value is
    # a placeholder.
    def q():
        return 0

    with tile.TileContext(nc) as tc:
        with (
            tc.tile_pool(name="const", bufs=1) as cpool,
            tc.tile_pool(name="sb", bufs=2) as sb,
            tc.tile_pool(name="idx", bufs=6) as idxp,
            tc.tile_pool(name="ag", bufs=ABUF) as agp,
            tc.tile_pool(name="gg", bufs=GBUF) as ggp,
            tc.tile_pool(name="sel", bufs=6) as selp,
            tc.tile_pool(name="cnt", bufs=3) as cntp,
            tc.tile_pool(name="x", bufs=6) as xp,
            tc.tile_pool(name="mlp", bufs=2) as mlpp,
            tc.tile_pool(name="pch", bufs=2, space="PSUM") as pch,
            tc.tile_pool(name="pc2", bufs=2, space="PSUM") as pc2,
            tc.tile_pool(name="pga", bufs=2, space="PSUM") as pga,
            tc.tile_pool(name="pms", bufs=2, space="PSUM") as pms,
        ):
            # ---- constants ----
            identf = cpool.tile([128, 128], F32)
            make_identity(nc, identf[:])
            identb = cpool.tile([128, 128], BF16)
            make_identity(nc, identb[:])
            wst = cpool.tile([128, 128], BF16)
            nc.sync.dma_start(out=wst[:], in_=wstack[:])
            if has_bias:
                bstk = cpool.tile([1, 128], BF16, tag="bstk", name="bstk")
                nc.sync.dma_start(out=bstk[:], in_=bstack[:])
                ones1 = cpool.tile([1, 128], BF16, tag="ones1", name="ones1")
                nc.vector.memset(ones1[:], 1.0)
            invs = cpool.tile([128, NBT], F32)
            nc.sync.dma_start(out=invs[:], in_=invseq[:])
            invn_t = cpool.tile([128, NBT], F32, tag="invn05", name="invn05")
            nc.sync.dma_start(out=invn_t[:], in_=invn05d[:])
            w1ts = [cpool.tile([128, 256], F32, tag=f"w1t{k}", name=f"w1t{k}") for k in range(3)]
            for k in range(3):
                lo, hi = k * 128, min((k + 1) * 128, 320)
                nc.sync.dma_start(out=w1ts[k][: hi - lo, :], in_=w1t[lo:hi, :])
            w2ts = [cpool.tile([128, 128], F32, tag=f"w2t{k}", name=f"w2t{k}") for k in range(2)]
            for k in range(2):
                nc.sync.dma_start(out=w2ts[k][:], in_=w2t[k * 128 : (k + 1) * 128, :])
            w3ts = cpool.tile([128, 1], F32)
            nc.sync.dma_start(out=w3ts[:], in_=w3t[:])
            b1s = [cpool.tile([128, 1], F32, tag=f"b1{k}", name=f"b1{k}") for k in range(2)]
            for k in range(2):
                nc.sync.dma_start(out=b1s[k][:], in_=b1d[k * 128 : (k + 1) * 128, None])
            b2s = cpool.tile([128, 1], F32)
            nc.sync.dma_start(out=b2s[:], in_=b2d[:, None])
            b3s = cpool.tile([1, 1], F32)
            nc.sync.dma_start(out=b3s[:], in_=b3d[:, None])
            t1res = cpool.tile([128, NCATE * D], BF16)
            if "c" in PARTS:
                nc.scalar.dma_start(out=t1res[:], in_=t1rest[:])

            # warmup gather: absorb the cold-start ucode IRAM load (k=0, q0)
            if "a" in PARTS or "g" in PARTS:
                wit = cpool.tile([128, 8], I16, tag="warmidx", name="warmidx")
                nc.sync.dma_start(out=wit[:], in_=gidx[0:128, 0:8])
                wdest = cpool.tile([128, 128], BF16, tag="warmdest", name="warmdest")
                nc.gpsimd.dma_gather(
                    out_ap=wdest[:].rearrange("p (s e) -> p s e", e=128),
                    in_ap=mem01b[0:SHG, :],
                    idxs_ap=wit[:],
                    num_idxs=128,
                    num_idxs_reg=128,
                    elem_size=128,
                    single_packet=False,
                    queue_num=0,
                )
                nc.sync.dma_start(out=warm[:, :], in_=wdest[0:1, 0:4])

            # user/item singles: per-(bt, piece) [128,1]-offset indirects (the
            # HW DGE only supports one offset per partition row). Queues are
            # normalized post-schedule along with the gathers.
            iuit = cpool.tile([128, NBT * 4], I32, tag="iuit", name="iuit")
            nc.sync.dma_start(out=iuit[:], in_=iui[:])
            UIall = cpool.tile([128, NBT * 4 * D], F32, tag="UIall", name="UIall")
            if "u" not in PARTS:
                nc.vector.memset(UIall[:], 0.0)
            UIs = {bt: UIall[:, bt * 4 * D : (bt + 1) * 4 * D] for bt in range(NBT)}

            def eui(bt):
                # singles for this bt; spread out so the cores-0/1 descriptor
                # work interleaves with gather generation instead of blocking
                # the head of the kernel
                if "u" not in PARTS:
                    return
                for j in range(bt * 4, (bt + 1) * 4):
                    nc.gpsimd.indirect_dma_start(
                        out=UIall[:, j * D : (j + 1) * D],
                        out_offset=None,
                        in_=taball[:],
                        in_offset=bass.IndirectOffsetOnAxis(
                            ap=iuit[:, j : j + 1], axis=0
                        ),
                    )

            gdests, gsels = {}, {}
            adests, asels = {}, {}
            gnns = {}

            def eg(bt):
                """Launch GNN gathers for bt (7 shard streams)."""
                for sh in range(NSH_G if "g" in PARTS else 0):
                    r0 = (bt * NSH_G + sh) * 128
                    it = idxp.tile([128, LG // 16], I16, tag="gidx", bufs=14)
                    nc.sync.dma_start(out=it[:], in_=gidx[r0 : r0 + 128, :])
                    sl = selp.tile([128, TG * 128], FP8, tag="gsel", bufs=14)
                    nc.scalar.dma_start(out=sl[:], in_=gsel[r0 : r0 + 128, :])
                    gsels[(bt, sh)] = sl
                    X = ggp.tile([128, TG * 128], BF16, tag="gdest", bufs=GBUF)
                    ng = gcnt[(bt, sh)]
                    nc.gpsimd.dma_gather(
                        out_ap=X[:, : ng].rearrange("p (s e) -> p s e", e=128),
                        in_ap=mem01b[sh * SHG : (sh + 1) * SHG, :],
                        idxs_ap=it[:, : ng // 16],
                        num_idxs=ng,
                        num_idxs_reg=ng,
                        elem_size=128,
                        single_packet=False,
                        queue_num=q(),
                    )
                    gdests[(bt, sh)] = X

            def ea(bt):
                """Launch adgroup gathers for bt (4 windows x 4 shards)."""
                aits = []
                for sh in range(NSH_A if "a" in PARTS else 0):
                    r0 = (bt * NSH_A + sh) * 128
                    it = idxp.tile([128, LA // 16], I16, tag="aidx", bufs=8)
                    nc.sync.dma_start(out=it[:], in_=aidx[r0 : r0 + 128, :])
                    aits.append(it)
                    sl = selp.tile([128, NW * TW * 32], FP8, tag="asel", bufs=8)
                    nc.scalar.dma_start(out=sl[:], in_=asel[r0 : r0 + 128, :])
                    asels[(bt, sh)] = sl
                for w in range(NW if "a" in PARTS else 0):
                    for sh in range(NSH_A):
                        dest = agp.tile([128, TW * 128], BF16, tag="adest", bufs=ABUF)
                        na = acnt[(bt, w, sh)]
                        nc.gpsimd.dma_gather(
                            out_ap=dest[:, : na].rearrange("p (s e) -> p s e", e=128),
                            in_ap=tab0b[sh * SHA : (sh + 1) * SHA, :],
                            idxs_ap=aits[sh][
                                :, w * (LW // 16) : w * (LW // 16) + na // 16
                            ],
                            num_idxs=na,
                            num_idxs_reg=na,
                            elem_size=128,
                            single_packet=False,
                            queue_num=q(),
                        )
                        adests[(bt, w, sh)] = dest

            def cg(bt):
                """Consume GNN gathers -> gnn[bt] (tanh(W x) aggregated)."""
                gnn = sb.tile([128, G], F32, tag="gnn", name=f"gnn{bt}", bufs=2)
                gnns[bt] = gnn
                if "g" not in PARTS:
                    nc.vector.memset(gnn[:], 0.0)
                    return
                nt = sum(gcnt[(bt, sh)] // 128 for sh in range(NSH_G))
                gaccA = pga.tile([128, 128], F32, tag="gacc", name=f"gaccA{bt}")
                gaccB = pga.tile([128, 128], F32, tag="gacc", name=f"gaccB{bt}")
                lastA = ((nt - 1) // 2) * 2
                lastB = ((nt - 2) // 2) * 2 + 1
                k = 0
                for sh in range(NSH_G):
                    X, sl = gdests[(bt, sh)], gsels[(bt, sh)]
                    tg_sh = gcnt[(bt, sh)] // 128
                    for t0 in range(0, tg_sh, 4):
                        gw = min(4, tg_sh - t0)
                        # transpose gw tiles [pos, feat] -> [feat, pos]
                        xt_ps = pch.tile(
                            [128, 4 * 128], BF16, tag="pchain", name=f"xt{bt}_{k}"
                        )
                        for i in range(gw):
                            nc.tensor.transpose(
                                out=xt_ps[:, i * 128 : (i + 1) * 128],
                                in_=X[:, (t0 + i) * 128 : (t0 + i + 1) * 128],
                                identity=identb[:],
                            )
                        xt = xp.tile([128, 4 * 128], BF16, tag="xt", bufs=3)
                        nc.vector.tensor_copy(
                            out=xt[:, : gw * 128], in_=xt_ps[:, : gw * 128]
                        )
                        for i in range(gw):
                            ps = pc2.tile([128, 128], F32, tag="pc2", name=f"ps{bt}_{k}")
                            if has_bias:
                                nc.tensor.matmul(
                                    ps[:], lhsT=ones1[:], rhs=bstk[:],
                                    start=True, stop=False,
                                )
                            nc.tensor.matmul(
                                ps[:],
                                lhsT=xt[:, i * 128 : (i + 1) * 128],
                                rhs=wst[:],
                                start=not has_bias,
                                stop=True,
                            )
                            h = xp.tile([128, 128], BF16, tag="h", bufs=6)
                            nc.scalar.activation(
                                out=h[:], in_=ps[:],
                                func=mybir.ActivationFunctionType.Tanh,
                            )
                            gacc = gaccA if k % 2 == 0 else gaccB
                            nc.tensor.matmul(
                                gacc[:],
                                lhsT=sl[:, (t0 + i) * 128 : (t0 + i + 1) * 128],
                                rhs=h[:],
                                start=k < 2,
                                stop=k in (lastA, lastB),
                            )
                            k += 1
                nc.vector.tensor_copy(out=gnn[:], in_=gaccA[:, :G])
                nc.vector.tensor_tensor(
                    out=gnn[:], in0=gnn[:], in1=gaccA[:, G:], op=mybir.AluOpType.add
                )
                nc.vector.tensor_tensor(
                    out=gnn[:], in0=gnn[:], in1=gaccB[:, :G], op=mybir.AluOpType.add
                )
                nc.vector.tensor_tensor(
                    out=gnn[:], in0=gnn[:], in1=gaccB[:, G:], op=mybir.AluOpType.add
                )
                nc.vector.tensor_scalar_mul(gnn[:], gnn[:], invn_t[:, bt : bt + 1])

            ftbs = {}

            def caW(bt):
                """Windows + cate + M + Pp + gnn-independent feature pieces.

                Runs EARLY on the tensor stream so the adgroup dest pool
                frees before the next bt's launches need it."""
                UI = UIs[bt]
                U, I = UI[:, : 2 * D], UI[:, 2 * D :]

                ftb = [
                    sb.tile([128, 128], F32, tag="ftb0", name=f"ftb0_{bt}"),
                    sb.tile([128, 128], F32, tag="ftb1", name=f"ftb1_{bt}"),
                    sb.tile([64, 128], F32, tag="ftb2", name=f"ftb2_{bt}"),
                ]
                ftbs[bt] = ftb
                for pi, piece in ((0, U), (1, I)):
                    p_ps = pms.tile([64, 128], F32, tag="pmisc", name=f"pt{bt}_{pi}")
                    nc.tensor.transpose(out=p_ps[:], in_=piece[:], identity=identf[:])
                    slab, row = divmod(pi * 64, 128)
                    nc.vector.tensor_copy(out=ftb[slab][row : row + 64, :], in_=p_ps[:])

                # window matmuls -> M
                M = sb.tile([128, 2 * D], F32, tag="M", name=f"M{bt}")
                if "a" not in PARTS:
                    nc.vector.memset(M[:], 0.0)
                for w in range(NW if "a" in PARTS else 0):
                    wps = pms.tile([32, 32], F32, tag="pmisc", name=f"wps{bt}_{w}")
                    nmm = sum(acnt[(bt, w, sh)] // 128 for sh in range(NSH_A))
                    k = 0
                    for sh in range(NSH_A):
                        sl = asels[(bt, sh)]
                        dest = adests[(bt, w, sh)]
                        for t in range(acnt[(bt, w, sh)] // 128):
                            nc.tensor.matmul(
                                wps[:],
                                lhsT=sl[:, (w * TW + t) * 32 : (w * TW + t + 1) * 32],
                                rhs=dest[:, t * 128 : t * 128 + 32],
                                start=(k == 0),
                                stop=(k == nmm - 1),
                            )
                            k += 1
                    nc.vector.tensor_copy(out=M[32 * w : 32 * w + 32, :D], in_=wps[:])

                # cate seq-sum (count-matmul against the resident cate table)
                cps = pms.tile([128, 32], F32, tag="pmisc", name=f"cps{bt}")
                for cg_i in range(5 if "c" in PARTS else 0):
                    c0, c1 = cg_i * 16, min((cg_i + 1) * 16, NCATE)
                    cs = cntp.tile([128, 16 * 128], FP8, tag="cnt", bufs=3)
                    nc.scalar.dma_start(
                        out=cs[:, : (c1 - c0) * 128],
                        in_=countst[:, bt * V1P + c0 * 128 : bt * V1P + c1 * 128],
                    )
                    for c in range(c0, c1):
                        nc.tensor.matmul(
                            cps[:],
                            lhsT=cs[:, (c - c0) * 128 : (c - c0 + 1) * 128],
                            rhs=t1res[:, c * D : (c + 1) * D],
                            start=(c == 0),
                            stop=(c == NCATE - 1),
                        )

                if "c" in PARTS:
                    nc.vector.tensor_copy(out=M[:, D:], in_=cps[:])
                else:
                    nc.vector.memset(M[:, D:], 0.0)
                nc.vector.tensor_scalar_mul(M[:], M[:], invs[:, bt : bt + 1])

                Pp = sb.tile([128, 2 * D], F32, tag="Pp")
                nc.vector.tensor_tensor(
                    out=Pp[:], in0=I[:], in1=M[:], op=mybir.AluOpType.mult
                )
                if DEBUG:
                    bsl = slice(bt * 128, (bt + 1) * 128)
                    for nm, tl in (("dbgU", U), ("dbgI", I), ("dbgM", M)):
                        nc.sync.dma_start(out=dbg[nm][bsl, :], in_=tl[:])

                # transpose remaining feature pieces (M, Pp) into fT tiles
                for pi, piece in ((2, M), (3, Pp)):
                    p_ps = pms.tile([64, 128], F32, tag="pmisc", name=f"pt{bt}_{pi}")
                    nc.tensor.transpose(out=p_ps[:], in_=piece[:], identity=identf[:])
                    slab, row = divmod(pi * 64, 128)
                    nc.vector.tensor_copy(out=ftb[slab][row : row + 64, :], in_=p_ps[:])

            def caT(bt):
                """gnn feature piece + MLP + output for bt (after cg(bt))."""
                bsl = slice(bt * 128, (bt + 1) * 128)
                gnn = gnns[bt]
                ftb = ftbs[bt]
                if DEBUG:
                    nc.sync.dma_start(out=dbg["dbgG"][bsl, :], in_=gnn[:])
                p_ps = pms.tile([64, 128], F32, tag="pmisc", name=f"pt{bt}_4")
                nc.tensor.transpose(out=p_ps[:], in_=gnn[:], identity=identf[:])
                slab, row = divmod(4 * 64, 128)
                nc.vector.tensor_copy(out=ftb[slab][row : row + 64, :], in_=p_ps[:])

                # per-bt MLP slice
                h1s = []
                for m in range(2):
                    h1_ps = pms.tile([128, 128], F32, tag="pmisc", name=f"h1ps{bt}_{m}")
                    for oi, kk in enumerate((0, 2, 1)):
                        kp = 128 if kk < 2 else 64
                        nc.tensor.matmul(
                            h1_ps[:],
                            lhsT=w1ts[kk][:kp, m * 128 : (m + 1) * 128],
                            rhs=ftb[kk][:kp, :],
                            start=(oi == 0),
                            stop=(oi == 2),
                        )
                    h1 = mlpp.tile([128, 128], F32, tag="h1", name=f"h1_{bt}_{m}")
                    nc.scalar.activation(
                        out=h1[:], in_=h1_ps[:],
                        func=mybir.ActivationFunctionType.Relu, bias=b1s[m][:, 0:1],
                    )
                    h1s.append(h1)
                h2_ps = pms.tile([128, 128], F32, tag="pmisc", name=f"h2ps{bt}")
                for m in range(2):
                    nc.tensor.matmul(
                        h2_ps[:], lhsT=w2ts[m][:], rhs=h1s[m][:],
                        start=(m == 0), stop=(m == 1),
                    )
                h2 = mlpp.tile([128, 128], F32, tag="h2", name=f"h2_{bt}")
                nc.scalar.activation(
                    out=h2[:], in_=h2_ps[:],
                    func=mybir.ActivationFunctionType.Relu, bias=b2s[:, 0:1],
                )
                lg_ps = pms.tile([1, 128], F32, tag="pmisc", name=f"lgps{bt}")
                nc.tensor.matmul(lg_ps[:], lhsT=w3ts[:], rhs=h2[:])
                lgt = mlpp.tile([1, 128], F32, tag="lg", name=f"lgt{bt}")
                nc.vector.tensor_scalar_add(lgt[:], lg_ps[:], b3s[:, 0:1])
                nc.sync.dma_start(out=out[None, bsl], in_=lgt[:])

            eui(0)
            eui(1)
            eui(2)
            eui(3)
            eg(0)
            ea(0)
            eg(1)
            caW(0)
            cg(0)
            ea(1)
            caT(0)
            eg(2)
            caW(1)
            cg(1)
            ea(2)
            caT(1)
            eg(3)
            caW(2)
            cg(2)
            ea(3)
            caT(2)
            cg(3)
            caW(3)
            caT(3)

    # Post-schedule queue assignment: the k-th Pool DMA (scheduled order)
    # uses DMASW lane k%8; each lane must stay on one SWDGE queue, so assign
    # queue = lane%4 to every Pool DMA (gathers via queue_num, indirect
    # copies via their queue name).
    pool_seq = []
    for bb in nc.m.functions[0].blocks:
        for inst in bb.instructions:
            nm = type(inst).__name__
            eng = getattr(inst, "engine", None)
            if nm == "InstDMAGatherAnt" or (
                nm == "InstDMACopy" and str(eng) == "EngineType.Pool"
            ):
                pool_seq.append((nm, inst))
    for k, (nm, inst) in enumerate(pool_seq):
        qq = (k % 8) % NQ
        if nm == "InstDMAGatherAnt":
            inst.queue_num = qq
        else:
            inst.queue = f"qPoolDynamic{qq or ''}"

    nc.compile()
    return nc


def _prep(inp):
    """Host-side input transforms -> per-core in_maps."""
    f32 = np.float32
    bf16 = mybir.dt.np(BF16)
    fp8 = mybir.dt.np(FP8)
    g = lambda k: np.asarray(inp[k])

    it0 = g("item_tab0").astype(f32)  # [100000, 32]
    it1 = g("item_tab1").astype(f32)  # [10000, 32]
    tab0b = np.zeros((NSH_A * SHA, 128), bf16)
    tab0b[: it0.shape[0], :D] = it0.astype(bf16)
    mem01 = np.concatenate([g("mem0"), g("mem1")], axis=1).astype(f32)
    mem01b = np.zeros((NSH_G * SHG, 128), bf16)
    mem01b[:VM] = mem01.astype(bf16)
    ut0 = np.asarray(g("user_tab0"), f32)
    ut1 = np.asarray(g("user_tab1"), f32)
    taball = np.zeros((NTAB, D), f32)
    taball[:50000] = ut0
    taball[50000:100000] = ut1
    taball[100000:200000] = it0
    taball[200000:210000] = it1
    tab1p = np.zeros((V1P, D), f32)
    tab1p[: it1.shape[0]] = it1
    t1rest = np.ascontiguousarray(
        tab1p.astype(bf16).reshape(NCATE, 128, D).transpose(1, 0, 2).reshape(128, NCATE * D)
    )

    wstack = np.zeros((128, 128), bf16)
    wstack[:G, :G] = g("W_agg0").T.astype(bf16)
    wstack[G:, G:] = g("W_agg1").T.astype(bf16)
    bstack = np.concatenate([g("b_agg0"), g("b_agg1")]).astype(f32)
    has_bias = bool(np.abs(bstack).max() > 0)
    w1t = np.ascontiguousarray(g("W1").T.astype(f32))
    w2t = np.ascontiguousarray(g("W2").T.astype(f32))
    w3t = np.ascontiguousarray(g("W3").T.astype(f32))
    b1 = g("b1").astype(f32); b2 = g("b2").astype(f32); b3 = g("b3").astype(f32)

    aseq = g("adgroup_id_seq").astype(np.int64)
    cseq = g("cate_id_seq").astype(np.int64)
    nbr = g("neighbor_ids").astype(np.int64)
    seq_mask = aseq != 0
    invseq_all = (1.0 / np.maximum(seq_mask.sum(-1), 1)).astype(f32)
    nmask = nbr != 0
    invn = (0.5 / np.maximum(nmask.sum(-1), 1)).astype(f32)

    def pack16(stream):
        # [L] -> [128, L//16]: idx k at [k%16, k//16], replicated x8
        w = stream.reshape(-1, 16).T.astype(np.int16)
        return np.tile(w, (8, 1))

    nl_a = {}  # (bt, w, sh) -> max nload over cores
    nl_g = {}  # (bt, sh) -> max nload over cores
    in_maps = []
    for c in range(NC):
        bs = slice(c * BC, (c + 1) * BC)
        a_c, c_c, m_c = aseq[bs], cseq[bs], seq_mask[bs]
        n_c = nbr[bs]
        invn_c = invn[bs]

        aidx_l = np.zeros((NBT * NSH_A * 128, LA // 16), np.int16)
        asel_l = np.zeros((NBT * NSH_A * 128, NW * TW * 32), fp8)
        gidx_l = np.zeros((NBT * NSH_G * 128, LG // 16), np.int16)
        gsel_l = np.zeros((NBT * NSH_G * 128, TG * 128), fp8)
        countst_l = np.zeros((128, NBT * V1P), fp8)
        iui_l = np.zeros((128, NBT * 4), np.int32)



        for bt in range(NBT):
            btsl = slice(bt * 128, (bt + 1) * 128)
            a = a_c[btsl]; cc = c_c[btsl]; mm = m_c[btsl]
            b_loc = np.repeat(np.arange(128), S)
            av = a.ravel(); mv = mm.ravel()
            b_m = b_loc[mv]; a_m = av[mv]
            sh_a = a_m // SHA; loc_a = a_m % SHA
            w_a = b_m // 32

            # adgroup streams: per (sh, w), dedup by loc with per-b counts
            for sh in range(NSH_A):
                stream4 = np.zeros(LA, np.int64)
                sel = np.zeros((128, NW * TW * 32), f32)
                for w in range(NW):
                    pick = (sh_a == sh) & (w_a == w)
                    ll, bb = loc_a[pick], b_m[pick] - 32 * w
                    key = ll * 32 + bb
                    uk, cnt = np.unique(key, return_counts=True)
                    lw, bw = uk // 32, uk % 32
                    uloc, inv = np.unique(lw, return_inverse=True)
                    nload = len(uloc)
                    assert nload <= LW, f"adgroup overflow {nload} > {LW}"
                    nl_a[(bt, w, sh)] = max(nl_a.get((bt, w, sh), 0), nload)
                    base = w * LW
                    stream4[base : base + nload] = uloc
                    stream4[base + nload : base + LW] = 0  # 0-pad (no trim)
                    pos = inv
                    sel[pos % 128, (w * TW + pos // 128) * 32 + bw] = cnt
                r0 = (bt * NSH_A + sh) * 128
                aidx_l[r0 : r0 + 128] = pack16(stream4)
                asel_l[r0 : r0 + 128] = sel.astype(fp8)

            # cate counts, pre-transposed: [128 p, (c,b)] = C[c*128+p, b]
            cm = cc.ravel()[mv]
            C = np.bincount(cm * 128 + b_m, minlength=V1P * 128).reshape(V1P, 128)
            countst_l[:, bt * V1P : (bt + 1) * V1P] = (
                C.reshape(NCATE, 128, 128).transpose(1, 0, 2).reshape(128, V1P).astype(fp8)
            )

            # GNN streams: per sh, dedup by loc with per-b weights
            nb = n_c[btsl]
            b_loc2 = np.repeat(np.arange(128), N)
            nv = nb.ravel()
            sh_g = nv // SHG; loc_g = nv % SHG
            for sh in range(NSH_G):
                pick = sh_g == sh
                ll, bb = loc_g[pick], b_loc2[pick]
                key = ll * 128 + bb
                uk, cnt = np.unique(key, return_counts=True)
                lw, bw = uk // 128, uk % 128
                wv = cnt * ((lw != 0) | (sh != 0))
                uloc, inv = np.unique(lw, return_inverse=True)
                nload = len(uloc)
                assert nload <= LG, f"gnn overflow {nload} > {LG}"
                nl_g[(bt, sh)] = max(nl_g.get((bt, sh), 0), nload)
                stream = np.zeros(LG, np.int64)  # 0-pad (no trim)
                stream[:nload] = uloc
                pos = inv
                sel = np.zeros((128, TG * 128), f32)
                sel[pos % 128, (pos // 128) * 128 + bw] = wv
                r0 = (bt * NSH_G + sh) * 128
                gidx_l[r0 : r0 + 128] = pack16(stream)
                gsel_l[r0 : r0 + 128] = sel.astype(fp8)

            # merged singles indices
            iui_l[:, bt * 4 + 0] = g("user_f0")[bs][btsl].astype(np.int32)
            iui_l[:, bt * 4 + 1] = 50000 + g("user_f1")[bs][btsl].astype(np.int32)
            iui_l[:, bt * 4 + 2] = 100000 + g("adgroup_id")[bs][btsl].astype(np.int32)
            iui_l[:, bt * 4 + 3] = 200000 + g("cate_id")[bs][btsl].astype(np.int32)

        in_maps.append(
            {
                "tab0b": tab0b, "mem01b": mem01b, "taball": taball,
                "t1rest": t1rest, "countst": countst_l, "iui": iui_l,
                "aidx": aidx_l, "gidx": gidx_l, "asel": asel_l, "gsel": gsel_l,
                "invseq": invseq_all[bs].reshape(NBT, 128).T.copy(),
                "invn05": invn_c.reshape(NBT, 128).T.copy().astype(f32),
                "wstack": wstack,
                "bstack": np.ascontiguousarray(bstack.astype(bf16)[None, :]),
                "w1t": w1t, "b1": b1, "w2t": w2t, "b2": b2, "w3t": w3t, "b3": b3,
            }
        )
    r128 = lambda n: max(128, -(-n // 128) * 128)
    acnt = {k: r128(v) for k, v in nl_a.items()}
    gcnt = {k: r128(v) for k, v in nl_g.items()}
    return in_maps, has_bias, acnt, gcnt


def kernel(**inputs) -> np.ndarray:
    in_maps, has_bias, acnt, gcnt = _prep(inputs)
    key = ("nc", has_bias, tuple(sorted(acnt.items())), tuple(sorted(gcnt.items())))
    if key not in _CACHE:
        _CACHE[key] = _build(has_bias, acnt, gcnt)
    nc = _CACHE[key]
    trace = bool(_os.environ.get("KERNEL_TRACE"))
    res = run_bass_kernel_spmd(nc, in_maps, list(range(NC)), trace=trace)
    _CACHE["last_result"] = res
    out = np.concatenate([res.results[c]["out"] for c in range(NC)])
    return out[:, None].astype(np.float32)
